# revision 27
# baseline (speedup 1.0000x reference)
"""Trainium2 Bass kernel for the DiffRenderer problem.

Math refactor (validated against the jax reference):
  The renderer's per-point MLP input collapses to
      a[b, pix, d, h] = depth[b, d] * g[b, pix, h] + e[b, h]
  with
      g[b] = Q @ V_b,  V_b = M_b^T @ W1[:3],  M_b = s_obj * R_obj
      e[b] = c_b @ W1[:3] + z_shape[b] @ W1[3:] + b1
      depth[b, d] = zs[d] * s_inv[b] + bb_depth[b]
  Layer 1 + relu:  h' = max(depth*g, -e)   (== relu(a) - e)
  Layer 2:         s  = W2 . h' + (W2 . e + b2)   (bias folded into tanh)
  sdf = tanh(s); then the zero-crossing depth extraction.

Sharding: 8 cores = 4 batches x 2 pixel-halves (2048 pixels/core, 64 depths).

Per-core device pipeline (all phases overlap via the Tile scheduler):
  PE:  g = Q @ V (float32r); 64x5 layer-2 matmuls with a sliding one-hot
       W2 stationary [128,64] so psum row d accumulates s[d, :] directly
       (float32r = 1 cycle/row); 16 transposes [64,128] -> [128,64]
  DVE: h' = max(depth_d * g, -e) for pixels 0..1407 (tensor_scalar
       mult+max, per-partition scalars); zero-crossing postprocess
  ACT: h' = relu(depth_d * g + e) for pixels 1408..2047 (activation with
       per-partition scale/bias); psum->sbuf copies; tanh (+bias fold)
Precision: float32r (FP22 in the PE) for g and layer-2, fp32 elsewhere;
  bf16 was measured to flip sdf signs in the randn-weight regime and is
  deliberately NOT used.
"""

import os
import sys

import numpy as np

for _p in ("/opt/trn_rl_repo", "/root/.axon_site/_ro/trn_rl_repo"):
    if os.path.isdir(_p) and _p not in sys.path:
        sys.path.append(_p)

from contextlib import ExitStack

from concourse import bacc, bass, masks, mybir, tile
from concourse.bass_utils import run_bass_kernel_spmd

F32 = mybir.dt.float32
F32R = mybir.dt.float32r
ALU = mybir.AluOpType
ACTF = mybir.ActivationFunctionType

IMG = 64
D = 64
HID = 128
BS = 4
NCORES = 8
PIX = IMG * IMG          # 4096 pixels per batch
PPC = PIX // 2           # 2048 pixels per core
NT = PPC // 128          # 16 pixel tiles per core
DVE_T = 11               # pixel tiles 0..DVE_T-1 computed on DVE, rest on ACT
SPLIT = DVE_T * 128      # 1024
K63 = 63                 # depth pairs per tile

_PROGRAM = None


def build_program():
    nc = bacc.Bacc(None, target_bir_lowering=False)
    qs = nc.declare_dram_parameter("qs", [3, PPC], F32R, isOutput=False)
    vb = nc.declare_dram_parameter("vb", [3, HID], F32R, isOutput=False)
    w2w = nc.declare_dram_parameter("w2w", [HID, 127], F32R, isOutput=False)
    depth = nc.declare_dram_parameter("depth", [128, D], F32, isOutput=False)
    nege = nc.declare_dram_parameter("nege", [128, 1], F32, isOutput=False)
    epos = nc.declare_dram_parameter("epos", [128, 1], F32, isOutput=False)
    bias0 = nc.declare_dram_parameter("bias0", [128, 1], F32, isOutput=False)
    bias1 = nc.declare_dram_parameter("bias1", [128, 1], F32, isOutput=False)
    zm1 = nc.declare_dram_parameter("zm1", [128, NT * K63], F32, isOutput=False)
    lam = nc.declare_dram_parameter("lam", [128, 1], F32, isOutput=False)
    dp_o = nc.declare_dram_parameter("dp", [128, NT], F32, isOutput=True)
    occ_o = nc.declare_dram_parameter("occ", [128, NT], F32, isOutput=True)

    with tile.TileContext(nc) as tc, ExitStack() as ctx:
        const = ctx.enter_context(tc.tile_pool(name="const", bufs=1))
        gpool = ctx.enter_context(tc.tile_pool(name="g", bufs=1))
        hpool = ctx.enter_context(tc.tile_pool(name="hp", bufs=4))
        spool = ctx.enter_context(tc.tile_pool(name="s", bufs=1))
        post = ctx.enter_context(tc.tile_pool(name="post", bufs=1))
        pss = ctx.enter_context(tc.tile_pool(name="pss", bufs=1, space="PSUM"))
        pst = ctx.enter_context(tc.tile_pool(name="pst", bufs=3, space="PSUM"))

        def load(handle, shape, dtype=F32, eng=None):
            nm = "t_" + handle.name
            t = const.tile(shape, dtype, name=nm, tag=nm)
            (eng or nc.sync).dma_start(t[:], handle[:])
            return t

        t_qs = load(qs, [3, PPC], F32R)
        t_vb = load(vb, [3, HID], F32R, eng=nc.gpsimd)
        t_depth = load(depth, [128, D], eng=nc.gpsimd)
        t_nege = load(nege, [128, 1], eng=nc.gpsimd)
        t_epos = load(epos, [128, 1], eng=nc.gpsimd)
        # tail-only constants: off the critical sync queue (SWDGE)
        def load_late(handle, shape):
            nm = "t_" + handle.name
            t = const.tile(shape, F32, name=nm, tag=nm)
            nc.gpsimd.dma_start(t[:], handle[:])
            return t
        t_bias0 = load_late(bias0, [128, 1])
        t_bias1 = load_late(bias1, [128, 1])
        t_zm1 = load_late(zm1, [128, NT * K63])
        t_lam = load_late(lam, [128, 1])
        ident = const.tile([64, 64], F32)
        masks.make_identity(nc, ident[:])
        # sliding-window one-hot W2 (host-built): col 63 holds W2, rest
        # zero; slice [:, 63-d : 127-d] puts W2 in stationary column d ->
        # matmul accumulates s[d, :] into psum partition d.
        w2win = load(w2w, [HID, 127], F32R)

        # ---- g = Q @ V  (float32r matmul; one-time) ----
        # psum slots shared (by tag) with the layer-2 chunk tiles: the g
        # phase finishes before the d-loop's accumulators are first used.
        g_sb = gpool.tile([HID, PPC], F32)
        for k in range(PPC // 512):
            pg = pss.tile([HID, 512], F32, name=f"pg{k}", tag=f"pss{k % 2}")
            nc.tensor.matmul(
                pg[:], t_vb[:], t_qs[:, k * 512:(k + 1) * 512],
                start=True, stop=True,
            )
            dst = g_sb[:, k * 512:(k + 1) * 512]
            if k % 2 == 0:
                nc.vector.tensor_copy(dst, pg[:])
            else:
                nc.scalar.copy(dst, pg[:])

        # ---- main d-loop: h' then layer-2 (accumulating masked matmuls) ----
        # chunk layout over the 2048 pixels (split DVE 1408 / ACT 640):
        CHUNKS = [(0, 512), (512, 512), (1024, 384), (1408, 384), (1792, 256)]
        ps_chunks = [
            pss.tile([D, n], F32, name=f"ps_chunk{k}", tag=f"pss{k}")
            for k, (off, n) in enumerate(CHUNKS)
        ]
        for d in range(D):
            hp_d = hpool.tile([HID, SPLIT], F32R, tag="hpd")
            hp_a = hpool.tile([HID, PPC - SPLIT], F32R, tag="hpa")
            nc.vector.tensor_scalar(
                hp_d[:], g_sb[:, 0:SPLIT],
                t_depth[:, d:d + 1], t_nege[:, 0:1],
                op0=ALU.mult, op1=ALU.max,
            )
            nc.scalar.activation(
                hp_a[:], g_sb[:, SPLIT:PPC], ACTF.Relu,
                bias=t_epos[:, 0:1], scale=t_depth[:, d:d + 1],
            )
            w2slice = w2win[:, 63 - d:127 - d]
            for k, (off, n) in enumerate(CHUNKS):
                if off >= SPLIT:
                    srck = hp_a[:, off - SPLIT:off - SPLIT + n]
                else:
                    srck = hp_d[:, off:off + n]
                nc.tensor.matmul(
                    ps_chunks[k][:], w2slice, srck,
                    start=(d == 0), stop=(d == D - 1),
                    skip_group_check=True,
                )

        # ---- tail: per-chunk copy (ACT), transpose, tanh, postprocess ----
        s_sb = spool.tile([D, PPC], F32)
        sdf = post.tile([128, NT * D], F32)
        pos = post.tile([128, NT * D], F32)
        zc = post.tile([128, NT * K63], F32)
        d1r = post.tile([128, NT], F32)
        s1 = post.tile([128, NT], F32)
        s2 = post.tile([128, NT], F32)
        sdf3 = sdf[:].rearrange("p (t d) -> p t d", d=D)
        pos3 = pos[:].rearrange("p (t d) -> p t d", d=D)
        zc3 = zc[:].rearrange("p (t k) -> p t k", k=K63)
        zm1_3 = t_zm1[:].rearrange("p (t k) -> p t k", k=K63)
        for k, (off, n) in enumerate(CHUNKS):
            if k < 2:
                nc.vector.tensor_copy(s_sb[:, off:off + n], ps_chunks[k][:])
            else:
                nc.scalar.copy(s_sb[:, off:off + n], ps_chunks[k][:])
            t0, t1 = off // 128, (off + n) // 128
            # transposes share one psum tile per chunk -> single batched tanh
            # (chunks 0-2 are all DVE-path tiles, 3-4 all ACT-path, so the
            # per-partition tanh bias is uniform within a chunk)
            pt = pst.tile([128, (t1 - t0) * D], F32, name=f"pt{k}", tag="pt", bufs=3)
            for j, i in enumerate(range(t0, t1)):
                nc.tensor.transpose(
                    pt[:, j * D:(j + 1) * D], s_sb[:, i * 128:(i + 1) * 128], ident[:])
            b_ap = t_bias0 if k < 3 else t_bias1
            nc.scalar.activation(
                sdf[:, t0 * D:t1 * D], pt[:, 0:(t1 - t0) * D], ACTF.Tanh,
                bias=b_ap[:, 0:1], scale=1.0,
            )

        # postprocess in 3 merged tile groups (chunk-aligned)
        for t0, t1 in ((0, 8), (8, 14), (14, 16)):
            ts_ = slice(t0, t1)
            nc.vector.tensor_scalar(
                pos[:, t0 * D:t1 * D],
                sdf[:, t0 * D:t1 * D], 0.0, None, op0=ALU.is_gt)
            nc.vector.scalar_tensor_tensor(
                zc3[:, ts_, :], pos3[:, ts_, 1:D], 0.5, pos3[:, ts_, 0:K63],
                op0=ALU.is_lt, op1=ALU.mult,
            )
            for qi, (in1, red_op, out) in enumerate((
                (zm1_3[:, ts_, :], ALU.min, d1r),
                (sdf3[:, ts_, 0:K63], ALU.add, s1),
                (sdf3[:, ts_, 1:D], ALU.add, s2),
            )):
                tmp = post.tile([128, 8 * K63], F32, tag=f"ppt{qi}", bufs=2)
                tmp3 = tmp[:, 0:(t1 - t0) * K63].rearrange("p (t k) -> p t k", k=K63)
                eng = nc.gpsimd if qi == 0 else nc.vector
                eng.tensor_tensor(tmp3, zc3[:, ts_, :], in1, op=ALU.mult)
                nc.vector.tensor_reduce(
                    out[:, ts_], tmp3, axis=mybir.AxisListType.X, op=red_op)

        # ---- global finals on [128, NT] ----
        occ_sb = post.tile([128, NT], F32)
        nc.vector.tensor_scalar(occ_sb[:], d1r[:], -50.0, None, op0=ALU.is_le)
        d1 = post.tile([128, NT], F32)
        nc.vector.tensor_scalar(d1[:], d1r[:], 100.0, None, op0=ALU.add)
        den = post.tile([128, NT], F32)
        nc.vector.scalar_tensor_tensor(
            den[:], s2[:], 1e-6, s1[:], op0=ALU.subtract, op1=ALU.subtract
        )
        rec = post.tile([128, NT], F32)
        nc.vector.reciprocal(rec[:], den[:])
        interp = post.tile([128, NT], F32)
        nc.vector.scalar_tensor_tensor(
            interp[:], rec[:], t_lam[:, 0:1], s1[:], op0=ALU.mult, op1=ALU.mult)
        res = post.tile([128, NT], F32)
        nc.vector.tensor_tensor(res[:], d1[:], interp[:], op=ALU.subtract)
        dp_sb = post.tile([128, NT], F32)
        nc.vector.tensor_tensor(dp_sb[:], occ_sb[:], res[:], op=ALU.mult)

        nc.sync.dma_start(dp_o[:], dp_sb[:])
        nc.sync.dma_start(occ_o[:], occ_sb[:])

    nc.finalize()
    return nc


def host_prep(z_shape, z_extr, W1, b1, W2, b2):
    """Per-core input maps. All small math mirrors the reference in
    float64 (deviations ~1e-7, far inside the sdf sign margins)."""
    f32 = np.float32
    z_shape = np.asarray(z_shape, f32)
    z_extr = np.asarray(z_extr, f32)
    W1 = np.asarray(W1, f32)
    b1 = np.asarray(b1, f32)
    W2 = np.asarray(W2, f32)
    b2 = np.asarray(b2, f32)

    f = 70.0 * (IMG / 64.0)
    cc = IMG / 2.0 - 0.5
    Km = np.array([[f, 0, cc], [0, f, cc], [0, 0, 1]], np.float64)
    K_inv = np.linalg.inv(Km)
    t = np.array([0.0, 0.0, 2.5])

    # mirror the reference's f32 double-reciprocal
    s_obj32 = (1.0 / z_extr[:, 0]).astype(f32)
    s_inv32 = (1.0 / s_obj32).astype(f32)
    s_obj = s_obj32.astype(np.float64)
    s_inv = s_inv32.astype(np.float64)
    t_obj = z_extr[:, 1:4].astype(np.float64)
    alpha = z_extr[:, 4].astype(np.float64)

    a = np.pi * alpha
    ca, sa = np.cos(a), np.sin(a)
    R_obj = np.zeros((BS, 3, 3))
    R_obj[:, 0, 0] = ca
    R_obj[:, 0, 1] = -sa
    R_obj[:, 1, 0] = sa
    R_obj[:, 1, 1] = ca
    R_obj[:, 2, 2] = 1.0

    corners = np.array(
        [[1, 1, 1], [1, 1, -1], [1, -1, 1], [1, -1, -1],
         [-1, 1, 1], [-1, 1, -1], [-1, -1, 1], [-1, -1, -1], [0, 0, 0]],
        np.float64,
    )
    R_obj_inv = np.linalg.inv(R_obj)
    # z-component of K @ (R_t^-1 (R_obj_inv (s_inv * corner) + t_obj) + t)
    zc = np.einsum("bij,aj->bai", R_obj_inv, corners)[:, :, 2] * s_inv[:, None]
    bb_depth = zc.mean(axis=1) + t_obj[:, 2] + 2.5      # (BS,)

    zs = np.linspace(-1.0, 1.0, D)
    depth_bd = (zs[None, :] * s_inv[:, None] + bb_depth[:, None]).astype(f32)

    M = s_obj[:, None, None] * R_obj
    c_b = np.einsum("bij,bj->bi", M, -(t[None, :] + t_obj))
    V = np.einsum("bij,ih->bjh", M, W1[:3].astype(np.float64))   # (BS,3,H)
    e = (
        np.einsum("bi,ih->bh", c_b, W1[:3].astype(np.float64))
        + z_shape.astype(np.float64) @ W1[3:].astype(np.float64)
        + b1.astype(np.float64)
    )
    e32 = e.astype(f32)
    s0 = (e32.astype(np.float64) @ W2.astype(np.float64) + b2.astype(np.float64))
    s0 = s0.astype(f32)                                  # (BS,1)

    xs = np.linspace(0.0, IMG - 1.0, IMG)
    Xg, Yg = np.meshgrid(xs, xs)
    p3 = np.stack([Xg.reshape(-1), Yg.reshape(-1), np.ones(PIX)], -1)
    q = p3 @ K_inv.T                                     # (PIX, 3)

    ones128 = np.ones((128, 1), f32)
    w2win_host = np.zeros((HID, 127), f32)
    w2win_host[:, 63] = W2[:, 0]
    in_maps = []
    for c in range(NCORES):
        b, half = c // 2, c % 2
        qs_c = q[half * PPC:(half + 1) * PPC].T.astype(f32)        # (3, PPC)
        vb_c = V[b].astype(f32)                                     # (3, H)
        dep = np.broadcast_to(depth_bd[b], (128, D)).astype(f32)
        zrow1 = np.tile(depth_bd[b][0:K63] - 100.0, NT).astype(f32)
        lam_b = np.float32(depth_bd[b][1] - depth_bd[b][0])
        in_maps.append({
            "qs": np.ascontiguousarray(qs_c),
            "vb": np.ascontiguousarray(vb_c),
            "w2w": w2win_host,
            "depth": np.ascontiguousarray(dep),
            "nege": (-e32[b]).reshape(HID, 1).astype(f32),
            "epos": e32[b].reshape(HID, 1).astype(f32),
            "bias0": (s0[b, 0] * ones128).astype(f32),
            "bias1": (b2[0] * ones128).astype(f32),
            "zm1": np.broadcast_to(zrow1, (128, NT * K63)).copy(),
            "lam": (lam_b * ones128).astype(f32),
        })
    return in_maps


def _assemble(results):
    f32 = np.float32
    dp_full = np.zeros((BS, PIX), f32)
    occ_full = np.zeros((BS, PIX), f32)
    for c in range(NCORES):
        b, half = c // 2, c % 2
        sl = slice(half * PPC, (half + 1) * PPC)
        dp_full[b, sl] = np.asarray(results[c]["dp"]).T.ravel()
        occ_full[b, sl] = np.asarray(results[c]["occ"]).T.ravel()
    return (
        dp_full.reshape(BS, IMG, IMG, 1),
        occ_full.reshape(BS, IMG, IMG, 1),
    )


def get_program():
    global _PROGRAM
    if _PROGRAM is None:
        _PROGRAM = build_program()
    return _PROGRAM


def kernel(z_shape, z_extr, W1, b1, W2, b2, **run_kwargs):
    nc = get_program()
    in_maps = host_prep(z_shape, z_extr, W1, b1, W2, b2)
    res = run_bass_kernel_spmd(nc, in_maps, core_ids=list(range(NCORES)), **run_kwargs)
    out = _assemble(res.results)
    if run_kwargs:
        return out, res
    return out
